# revision 11
# baseline (speedup 1.0000x reference)
# Self-attention kernel for Trainium2 (Bass/Tile), SPMD over 8 NeuronCores.
#
# Problem: x [8, 16, 2048, 128] f32; per (batch, series) block B = x[b, s]:
#   S = B @ B.T * SCALE            ([2048, 2048])
#   P = softmax(S, axis=-1)
#   out = P @ B                    ([2048, 128])
# 128 independent blocks, data-parallel: core i handles batch i (16 blocks).
#
# Per-block algorithm (flash-style, no S materialization in HBM):
#   - xn:  x block in natural layout [128p, 16c, 129] bf16 (col 128 = ones)
#   - xT:  x block transposed [d=128p, w=2048] bf16 (via DMA xbar transpose)
#   - for each w-block (1024 queries), for each u-chunk (128 keys):
#       S^T chunk [u=128, w=1024] = xT[:,uchunk].T @ xT[:,wblock]  (2 MMs, PSUM)
#       E^T = exp(SCALE * S^T)  (ScalarE, PSUM->SBUF bf16)
#       for each w-tile (128): psum_o[wtile] += E^T[:,wtile].T @ [x_chunk | 1]
#         (N=129: col 128 accumulates the softmax denominator)
#   - epilogue: out[wtile] = psum_o[:, :128] * (1 / psum_o[:, 128])  (VectorE)
#
# Softmax max-subtraction is skipped: logits are S * 1/512 with |S| ~ 60 max,
# so exp() arguments are within [-0.2, 0.2] -- no overflow possible, and
# softmax is shift-invariant so the result matches the reference.

import numpy as np

W = 2048  # input window (queries/keys per block)
D = 128  # head dim
C = 16  # u-chunks of 128 per block
WBS = 512  # queries per w-block pass (4 w-tiles, one PSUM bank each)
SCALE = 1.0 / ((W * D) ** 0.5)  # tau = 1.0

N_CORES = 8
BLOCKS_PER_CORE = 16

# exp(x*SCALE) ~= (1 + x*(A + x^2*CC) + x^2*B)^2  -- max rel err 5.3e-6 over
# |x| <= 90 (raw logits are N(0, 11.3^2), so |x| < ~70).  Evaluated as a
# single custom DVE op so VectorE can take a share of the softmax exp work
# that otherwise saturates ScalarE.
EXP_A = SCALE / 2
EXP_B = SCALE**2 / 8
EXP_CC = SCALE**3 / 48

# which ui iterations (of 8 per w-block) run exp on VectorE instead of ScalarE
DVE_UI = (2, 4, 6)

_nc_cache = {}
_exp_op_cache = []


def _register_exp_poly():
    """Register the EXP_POLY_ANT custom DVE op (idempotent)."""
    if _exp_op_cache:
        return _exp_op_cache[0]
    import numpy as np_
    from concourse import dve_ops
    from concourse.dve_spec import Spec, Src0, C0, C1, C2, One, sq

    x2 = sq(Src0)
    spec = Spec(
        body=sq(One + Src0 * (C0 + x2 * C2) + x2 * C1),
        reference=lambda in0, in1, s0, s1, imm2: (
            (1.0 + in0 * (s0 + in0 * in0 * imm2) + in0 * in0 * s1) ** 2
        ).astype(np_.float32),
    )
    op = dve_ops.DveOp(
        "EXP_POLY_ANT",
        spec,
        subdim=False,
        uops_sha={"v3": "b50dd4e6efd3c970"},
    )
    if op.name not in dve_ops._SUB_OPCODE_FOR_NAME:
        dve_ops.OPS.append(op)
        dve_ops._SUB_OPCODE_FOR_NAME[op.name] = (
            dve_ops._CUSTOM_DVE_ROW_BASE + len(dve_ops.OPS) - 1
        )
        dve_ops.CUSTOM_DVE_SPECS[op.name] = spec
    _exp_op_cache.append(op)
    return op


def _build_nc(n_blocks: int, passes: int = 1):
    from contextlib import ExitStack

    import concourse.tile as tile
    from concourse import bacc, mybir

    f32 = mybir.dt.float32
    bf16 = mybir.dt.float16  # fp16: same matmul/LDW speed as bf16, 4x mantissa
    Exp = mybir.ActivationFunctionType.Exp
    exp_op = _register_exp_poly()

    nc = bacc.Bacc(
        "TRN2", target_bir_lowering=False, debug=False, num_devices=N_CORES
    )
    x = nc.dram_tensor("x", [n_blocks, W, D], f32, kind="ExternalInput").ap()
    y = nc.dram_tensor("y", [n_blocks, W, D], f32, kind="ExternalOutput").ap()

    with tile.TileContext(nc) as tc, ExitStack() as ctx:
        xf_pool = ctx.enter_context(tc.tile_pool(name="xf", bufs=2))
        xn_pool = ctx.enter_context(tc.tile_pool(name="xn", bufs=2))
        xt_pool = ctx.enter_context(tc.tile_pool(name="xt", bufs=2))
        et_pool = ctx.enter_context(tc.tile_pool(name="et", bufs=3))
        out_pool = ctx.enter_context(tc.tile_pool(name="outp", bufs=8))
        r_pool = ctx.enter_context(tc.tile_pool(name="rp", bufs=8))
        ps_pool = ctx.enter_context(tc.tile_pool(name="ps", bufs=2, space="PSUM"))
        po_pool = ctx.enter_context(tc.tile_pool(name="po", bufs=1, space="PSUM"))

        for blk in [b for _ in range(passes) for b in range(n_blocks)]:
            # ---- load & prep ----
            xn_f32 = xf_pool.tile([128, C, D + 1], f32)
            nc.sync.dma_start(
                out=xn_f32[:, :, 0:D],
                in_=x[blk].rearrange("(c p) d -> p c d", p=128),
            )
            nc.gpsimd.memset(xn_f32[:, :, D : D + 1], 1.0)
            xn16 = xn_pool.tile([128, C, D + 1], bf16)
            nc.vector.tensor_copy(out=xn16, in_=xn_f32)
            xT16 = xt_pool.tile([128, W], bf16)
            for c in range(C):
                nc.sync.dma_start_transpose(
                    xT16[:, c * 128 : (c + 1) * 128], xn16[:, c, 0:128]
                )

            # ---- attention ----
            n_wt = WBS // 128  # w-tiles per pass (4)
            for wb in range(W // WBS):
                # one PSUM bank per w-tile: accumulation group stays open
                # across the whole u loop (start=True zeroes a full bank)
                psum_o = po_pool.tile([128, n_wt, 512], f32)
                for ui in range(C // 2):
                    # S^T for two u-chunks side by side -> one wide exp ACT
                    psum_s = ps_pool.tile([128, 2 * WBS], f32)
                    for uu in range(2):
                        u = 2 * ui + uu
                        nc.tensor.matmul(
                            psum_s[:, uu * WBS : (uu + 1) * WBS],
                            lhsT=xT16[:, u * 128 : (u + 1) * 128],
                            rhs=xT16[:, wb * WBS : (wb + 1) * WBS],
                            start=True,
                            stop=True,
                        )
                    et = et_pool.tile([128, 2 * WBS], bf16)
                    if ui in DVE_UI:
                        nc.vector._custom_dve(
                            exp_op,
                            out=et,
                            in0=psum_s,
                            s0=EXP_A,
                            s1=EXP_B,
                            imm2=EXP_CC,
                        )
                    else:
                        nc.scalar.activation(
                            out=et, in_=psum_s, func=Exp, scale=SCALE
                        )
                    for uu in range(2):
                        u = 2 * ui + uu
                        for wi in range(n_wt):
                            nc.tensor.matmul(
                                psum_o[:, wi, 0 : D + 1],
                                lhsT=et[:, uu * WBS + wi * 128 : uu * WBS + (wi + 1) * 128],
                                rhs=xn16[:, u, :],
                                start=(u == 0),
                                stop=(u == C - 1),
                            )
                # ---- normalize + store ----
                # per-wi so each PSUM bank releases as soon as its own
                # accumulation group closes (pipelines into the next pass)
                for wi in range(n_wt):
                    rcol = r_pool.tile([128, 1], f32)
                    nc.vector.reciprocal(
                        out=rcol, in_=psum_o[:, wi, D : D + 1]
                    )
                    ot = out_pool.tile([128, D], f32)
                    nc.vector.tensor_scalar_mul(
                        out=ot, in0=psum_o[:, wi, 0:D], scalar1=rcol
                    )
                    w0 = wb * WBS + wi * 128
                    nc.sync.dma_start(out=y[blk, w0 : w0 + 128, :], in_=ot)

    nc.compile()
    return nc


def _get_nc(n_blocks: int):
    if n_blocks not in _nc_cache:
        _nc_cache[n_blocks] = _build_nc(n_blocks)
    return _nc_cache[n_blocks]


def kernel(x: np.ndarray) -> np.ndarray:
    from concourse import bass_utils

    x = np.ascontiguousarray(np.asarray(x), dtype=np.float32)
    B, S, Wx, Dx = x.shape
    assert (B, S, Wx, Dx) == (N_CORES, BLOCKS_PER_CORE, W, D), x.shape

    nc = _get_nc(S)
    in_maps = [{"x": x[i]} for i in range(N_CORES)]
    res = bass_utils.run_bass_kernel_spmd(nc, in_maps, core_ids=list(range(N_CORES)))
    out = np.stack([res.results[i]["y"] for i in range(N_CORES)], axis=0)
    return out.astype(np.float32)


# revision 12
# speedup vs baseline: 1.3012x; 1.3012x over previous
# Self-attention kernel for Trainium2 (Bass/Tile), SPMD over 8 NeuronCores.
#
# Problem: x [8, 16, 2048, 128] f32; per (batch, series) block B = x[b, s]:
#   S = B @ B.T * SCALE            ([2048, 2048])
#   P = softmax(S, axis=-1)
#   out = P @ B                    ([2048, 128])
# 128 independent blocks, data-parallel: core i handles batch i (16 blocks).
#
# Per-block algorithm (flash-style, no S materialization in HBM):
#   - xn:  x block in natural layout [128p, 16c, 129] bf16 (col 128 = ones)
#   - xT:  x block transposed [d=128p, w=2048] bf16 (via DMA xbar transpose)
#   - for each w-block (1024 queries), for each u-chunk (128 keys):
#       S^T chunk [u=128, w=1024] = xT[:,uchunk].T @ xT[:,wblock]  (2 MMs, PSUM)
#       E^T = exp(SCALE * S^T)  (ScalarE, PSUM->SBUF bf16)
#       for each w-tile (128): psum_o[wtile] += E^T[:,wtile].T @ [x_chunk | 1]
#         (N=129: col 128 accumulates the softmax denominator)
#   - epilogue: out[wtile] = psum_o[:, :128] * (1 / psum_o[:, 128])  (VectorE)
#
# Softmax max-subtraction is skipped: logits are S * 1/512 with |S| ~ 60 max,
# so exp() arguments are within [-0.2, 0.2] -- no overflow possible, and
# softmax is shift-invariant so the result matches the reference.

import numpy as np

W = 2048  # input window (queries/keys per block)
D = 128  # head dim
C = 16  # u-chunks of 128 per block
WBS = 512  # queries per w-block pass (4 w-tiles, one PSUM bank each)
SCALE = 1.0 / ((W * D) ** 0.5)  # tau = 1.0

N_CORES = 8
BLOCKS_PER_CORE = 16

# exp(x*SCALE) ~= (1 + x*(A + x^2*CC) + x^2*B)^2  -- max rel err 5.3e-6 over
# |x| <= 90 (raw logits are N(0, 11.3^2), so |x| < ~70).  Evaluated as a
# single custom DVE op so VectorE can take a share of the softmax exp work
# that otherwise saturates ScalarE.
EXP_A = SCALE / 2
EXP_B = SCALE**2 / 8
EXP_CC = SCALE**3 / 48

# which ui iterations (of 8 per w-block) run exp on VectorE instead of ScalarE
DVE_UI = (2, 4, 6)
# engine issuing input-load + output-store DMAs ("sync" or "scalar");
# transposes always go on the other hwdge ring when split
IO_DMA_ENGINE = "sync"
# normalize-mul engine: "vector" (DVE tensor_scalar) or "scalar" (ACT copy+scale)
NORM_ENGINE = "vector"

_nc_cache = {}
_exp_op_cache = []


def _register_exp_poly():
    """Register the EXP_POLY_ANT custom DVE op (idempotent)."""
    if _exp_op_cache:
        return _exp_op_cache[0]
    import numpy as np_
    from concourse import dve_ops
    from concourse.dve_spec import Spec, Src0, C0, C1, C2, One, sq

    x2 = sq(Src0)
    spec = Spec(
        body=sq(One + Src0 * (C0 + x2 * C2) + x2 * C1),
        reference=lambda in0, in1, s0, s1, imm2: (
            (1.0 + in0 * (s0 + in0 * in0 * imm2) + in0 * in0 * s1) ** 2
        ).astype(np_.float32),
    )
    op = dve_ops.DveOp(
        "EXP_POLY_ANT",
        spec,
        subdim=False,
        uops_sha={"v3": "b50dd4e6efd3c970"},
    )
    if op.name not in dve_ops._SUB_OPCODE_FOR_NAME:
        dve_ops.OPS.append(op)
        dve_ops._SUB_OPCODE_FOR_NAME[op.name] = (
            dve_ops._CUSTOM_DVE_ROW_BASE + len(dve_ops.OPS) - 1
        )
        dve_ops.CUSTOM_DVE_SPECS[op.name] = spec
    _exp_op_cache.append(op)
    return op


def _build_nc(n_blocks: int, passes: int = 1):
    from contextlib import ExitStack

    import concourse.tile as tile
    from concourse import bacc, mybir

    f32 = mybir.dt.float32
    bf16 = mybir.dt.float16  # fp16: same matmul/LDW speed as bf16, 4x mantissa
    Exp = mybir.ActivationFunctionType.Exp
    exp_op = _register_exp_poly()

    nc = bacc.Bacc(
        "TRN2", target_bir_lowering=False, debug=False, num_devices=N_CORES
    )
    x = nc.dram_tensor("x", [n_blocks, W, D], f32, kind="ExternalInput").ap()
    y = nc.dram_tensor("y", [n_blocks, W, D], f32, kind="ExternalOutput").ap()

    with tile.TileContext(nc) as tc, ExitStack() as ctx:
        xf_pool = ctx.enter_context(tc.tile_pool(name="xf", bufs=2))
        xn_pool = ctx.enter_context(tc.tile_pool(name="xn", bufs=2))
        xt_pool = ctx.enter_context(tc.tile_pool(name="xt", bufs=2))
        et_pool = ctx.enter_context(tc.tile_pool(name="et", bufs=3))
        out_pool = ctx.enter_context(tc.tile_pool(name="outp", bufs=8))
        r_pool = ctx.enter_context(tc.tile_pool(name="rp", bufs=8))
        ps_pool = ctx.enter_context(tc.tile_pool(name="ps", bufs=2, space="PSUM"))
        po_pool = ctx.enter_context(tc.tile_pool(name="po", bufs=1, space="PSUM"))

        for blk in [b for _ in range(passes) for b in range(n_blocks)]:
            # ---- load & prep ----
            xn_f32 = xf_pool.tile([128, C, D + 1], f32)
            nc.sync.dma_start(
                out=xn_f32[:, :, 0:D],
                in_=x[blk].rearrange("(c p) d -> p c d", p=128),
            )
            nc.gpsimd.memset(xn_f32[:, :, D : D + 1], 1.0)
            xn16 = xn_pool.tile([128, C, D + 1], bf16)
            nc.vector.tensor_copy(out=xn16, in_=xn_f32)
            xT16 = xt_pool.tile([128, W], bf16)
            for c in range(C):
                nc.sync.dma_start_transpose(
                    xT16[:, c * 128 : (c + 1) * 128], xn16[:, c, 0:128]
                )

            # ---- attention ----
            n_wt = WBS // 128  # w-tiles per pass (4)
            for wb in range(W // WBS):
                # one PSUM bank per w-tile: accumulation group stays open
                # across the whole u loop (start=True zeroes a full bank)
                psum_o = po_pool.tile([128, n_wt, 512], f32)
                for ui in range(C // 2):
                    # S^T for two u-chunks side by side -> one wide exp ACT
                    psum_s = ps_pool.tile([128, 2 * WBS], f32)
                    for uu in range(2):
                        u = 2 * ui + uu
                        nc.tensor.matmul(
                            psum_s[:, uu * WBS : (uu + 1) * WBS],
                            lhsT=xT16[:, u * 128 : (u + 1) * 128],
                            rhs=xT16[:, wb * WBS : (wb + 1) * WBS],
                            start=True,
                            stop=True,
                        )
                    et = et_pool.tile([128, 2 * WBS], bf16)
                    if ui in DVE_UI:
                        nc.vector._custom_dve(
                            exp_op,
                            out=et,
                            in0=psum_s,
                            s0=EXP_A,
                            s1=EXP_B,
                            imm2=EXP_CC,
                        )
                    else:
                        nc.scalar.activation(
                            out=et, in_=psum_s, func=Exp, scale=SCALE
                        )
                    for uu in range(2):
                        u = 2 * ui + uu
                        for wi in range(n_wt):
                            nc.tensor.matmul(
                                psum_o[:, wi, 0 : D + 1],
                                lhsT=et[:, uu * WBS + wi * 128 : uu * WBS + (wi + 1) * 128],
                                rhs=xn16[:, u, :],
                                start=(u == 0),
                                stop=(u == C - 1),
                            )
                # ---- normalize + store ----
                # per-wi so each PSUM bank releases as soon as its own
                # accumulation group closes (pipelines into the next pass)
                for wi in range(n_wt):
                    rcol = r_pool.tile([128, 1], f32)
                    nc.vector.reciprocal(
                        out=rcol, in_=psum_o[:, wi, D : D + 1]
                    )
                    ot = out_pool.tile([128, D], f32)
                    nc.vector.tensor_scalar_mul(
                        out=ot, in0=psum_o[:, wi, 0:D], scalar1=rcol
                    )
                    w0 = wb * WBS + wi * 128
                    nc.sync.dma_start(out=y[blk, w0 : w0 + 128, :], in_=ot)

    nc.compile()
    return nc


def _get_nc(n_blocks: int):
    if n_blocks not in _nc_cache:
        _nc_cache[n_blocks] = _build_nc(n_blocks)
    return _nc_cache[n_blocks]


def kernel(x: np.ndarray) -> np.ndarray:
    from concourse import bass_utils

    x = np.ascontiguousarray(np.asarray(x), dtype=np.float32)
    B, S, Wx, Dx = x.shape
    assert (B, S, Wx, Dx) == (N_CORES, BLOCKS_PER_CORE, W, D), x.shape

    nc = _get_nc(S)
    in_maps = [{"x": x[i]} for i in range(N_CORES)]
    res = bass_utils.run_bass_kernel_spmd(nc, in_maps, core_ids=list(range(N_CORES)))
    out = np.stack([res.results[i]["y"] for i in range(N_CORES)], axis=0)
    return out.astype(np.float32)
